# revision 1
# baseline (speedup 1.0000x reference)
"""DeformationLoss kernel for 8 Trainium2 NeuronCores.

Math: loss = (1/num_pairs) * sum_{i<j} mean_k || d_i,k - d_j,k ||_2,
with d = pred - recon, B=512, J=32.

Strategy: shard the 32 joints across 8 cores (4 joints each); every core
computes the upper-triangle (block granularity 128) of the 512x512
pairwise squared distances for its joints via K=9 bf16 matmuls:

    h[i,j] = g_ij - (n_i + n_j)/2,  via rows
        L = [1,1,1, dx,dy,dz, m1,m2,m3],  R = [m1,m2,m3, dx,dy,dz, 1,1,1]
    where m1+m2+m3 is a 3-way bf16 split of -n/2 (n = ||d_i||^2 from the
    bf16-rounded d), making the diagonal h_ii ~ 0 to fp32 roundoff.

Diagonal 128x128 blocks use half-scaled operands (h/4), so their sqrt
contributes deform/2; off-diagonal upper blocks contribute deform once.
Host multiplies the grand total by 2: diagonal blocks count once and
each off-diag pair twice (i<j and j>i), reproducing the full-matrix sum.
ScalarE does deform = sqrt(-2*h + EPS) with a fused per-partition
running sum (accum_out): each pairwise tile is read exactly once.
Host sums the 8x[128,4] partials in f64.
"""

import numpy as np

B, J, C = 512, 32, 3
NCORES = 8
J_LOC = J // NCORES  # joints per core
NUM_PAIRS = B * (B - 1) // 2
EPS = 1e-5
N_DUMMY_MM = 7  # PE HAM warm-up matmuls issued during prep

_STATE = {}


def _ensure_path():
    import sys
    try:
        import concourse.bass  # noqa: F401
    except ImportError:
        for p in ("/opt/trn_rl_repo", "/root/.axon_site/_ro/trn_rl_repo"):
            if p not in sys.path:
                sys.path.insert(0, p)


def _split_multi_waits_json(bir_json: bytes) -> bytes:
    """The walrus in this image rejects instructions carrying >1 sync wait
    ("Too many sync wait commands", CoreV3GenImpl setupSyncWait).  Tile's
    scheduler emits such instructions (notably the kernel-tail Drain).
    Rewrite the BIR: hoist all-but-the-last wait of each instruction into
    dedicated single-wait NoOps right before it on the same engine."""
    import orjson

    d = orjson.loads(bir_json)
    changed = False
    for fn in d.get("functions", []):
        for bb in fn.get("blocks", []):
            out = []
            for ins in bb.get("instructions", []):
                si = ins.get("sync_info")
                waits = (si or {}).get("on_wait") or []
                if len(waits) > 1:
                    changed = True
                    for i, w in enumerate(waits[:-1]):
                        out.append({
                            "debug": ins.get("debug", 0),
                            "engine": ins["engine"],
                            "ins": [],
                            "name": f"{ins['name']}-hw{i}",
                            "opcode": "NoOp",
                            "outs": [],
                            "sync_info": {"on_update": [], "on_wait": [w]},
                        })
                    si["on_wait"] = [waits[-1]]
                out.append(ins)
            bb["instructions"] = out
    if not changed:
        return bir_json
    return orjson.dumps(d)


def install_walrus_wait_split(max_sem_num: int | None = 176):
    """Monkeypatch compile_bir_kernel so every bass compile in this process
    goes through the multi-wait splitter; optionally cap walrus's semaphore
    space so its NEFF epilogue clears fewer semaphores (the stock epilogue
    zeroes all 253 one-by-one, ~6.5us of tail)."""
    _ensure_path()
    import concourse.bass_utils as bu
    import concourse.bass2jax as b2j

    if getattr(bu, "_wait_split_installed", False):
        return
    orig = bu.compile_bir_kernel

    def patched(bir_json, tmpdir, neff_name="file.neff"):
        return orig(_split_multi_waits_json(bytes(bir_json)), tmpdir, neff_name)

    bu.compile_bir_kernel = patched
    b2j.compile_bir_kernel = patched

    if max_sem_num is not None:
        orig_args = bu.get_walrus_args

        def patched_args(*a, **k):
            return orig_args(*a, **k) + [f"--max-sem-num={max_sem_num}"]

        bu.get_walrus_args = patched_args
    bu._wait_split_installed = True


def _install_cheap_tile_teardown():
    """Replace TileContext's expensive tail (drain + all-engine barrier +
    sem clears + barrier, ~3us) with a single SP drain that waits on the
    global clock.  Safe here because the NEFF epilogue emitted by walrus
    unconditionally zeroes every semaphore and runs its own all-engine
    barrier, and bass's preamble re-clears the kernel sem range + DMA
    queues at the start of every execution."""
    import concourse.tile as tile
    from concourse.vector_clock import ScopedClock

    if getattr(tile.TileContext, "_cheap_teardown", False):
        return

    def _drain_and_barrier(self, tick_clock, wait_clock):
        # Emit no tail synchronization at all.  The NEFF epilogue runs an
        # all-engine barrier followed by ~6us of semaphore clears before the
        # program can complete, which dwarfs the output DMA's ~0.8us
        # completion latency — the data is long since landed by the time the
        # NEFF can possibly finish.
        popped = self.nc._tile_sem_poison_stack.pop()
        assert popped is self._sem_poison

    tile.TileContext._drain_and_barrier = _drain_and_barrier
    tile.TileContext._cheap_teardown = True


def build_bass():
    """Build the (uniform) single-core Bass program."""
    _ensure_path()
    import concourse.bass as bass
    import concourse.tile as tile
    from concourse import mybir
    from concourse.masks import make_identity

    _install_cheap_tile_teardown()

    f32 = mybir.dt.float32
    bf16 = mybir.dt.bfloat16
    SUB = mybir.AluOpType.subtract

    nc = bass.Bass()
    x = nc.dram_tensor("x", [128, 96], f32, kind="ExternalInput")
    acc_out = nc.dram_tensor("acc", [128, J_LOC + 1], f32, kind="ExternalOutput")

    with tile.TileContext(nc) as tc:
        with (
            tc.tile_pool(name="sb", bufs=1) as sb,
            tc.tile_pool(name="ps", bufs=2, space="PSUM") as ps,
        ):
            # x[p, t*48 + ci*12 + kl*3 + c] = (pred|recon)[128*ci + p, k0+kl, c]
            # Split the load across 3 DMA queues (sync/scalar HWDGE + gpsimd
            # SWDGE) — a single queue moves the 48KB in ~2.6us.
            X = sb.tile([128, 96], f32)
            for eng, p0, p1 in ((nc.sync, 0, 64), (nc.scalar, 64, 128)):
                eng.dma_start(out=X[p0:p1, :], in_=x[p0:p1, :])

            # Per-partition epsilon bias for sqrt(-2h + EPS).
            eps_t = sb.tile([128, 1], f32)
            nc.vector.memset(eps_t[:, :], EPS)

            # Warm the ACT sqrt table set early so the ~2.7us table load
            # overlaps the prep phase instead of stalling the first real op.
            warm = sb.tile([1, 1], f32)
            nc.vector.memset(warm, 0.0)
            nc.scalar.activation(
                warm, warm, mybir.ActivationFunctionType.Sqrt, bias=eps_t[:1, :], scale=1.0
            )

            # (No PE warm-up matmuls: on this device the PE clock gate stays
            # at 1.2 GHz regardless of activity, so dummies only delay the
            # transposes behind them.)
            DB = sb.tile([128, 48], bf16)  # bf16(d)
            nc.vector.tensor_tensor(out=DB[:, :], in0=X[:, 0:48], in1=X[:, 48:96], op=SUB)

            SQ = sb.tile([128, 48], f32)  # exact fp32 products of bf16 d
            nc.vector.tensor_mul(SQ[:, :], DB[:, :], DB[:, :])
            # NN = -n = -sum_c d_c^2 ; M = -n/2 (exact halving, stays on DVE)
            NN = sb.tile([128, 16], f32)
            nc.vector.tensor_reduce(
                out=NN[:, :],
                in_=SQ.rearrange("p (k c) -> p k c", c=3),
                axis=mybir.AxisListType.X,
                op=mybir.AluOpType.add,
                negate=True,
            )
            # The "ones" rows of the W stacks hold 0.5, so the matmul pairs
            # contribute 0.5*(m_i + m_j) with m = -n directly — no extra
            # halving op.  3-way bf16 split of m = -n, packed in one tile:
            # M123 cols [0:16)=m1, [16:32)=m2, [32:48)=m3
            M = NN
            M123 = sb.tile([128, 48], bf16)
            M1 = M123[:, 0:16]
            M2 = M123[:, 16:32]
            M3 = M123[:, 32:48]
            nc.vector.tensor_copy(M1, M[:, :])
            # M2 = bf16(M - M1) fused via bf16 output; R1 recomputed on gpsimd
            nc.vector.tensor_tensor(out=M2, in0=M[:, :], in1=M1, op=SUB)
            R1 = sb.tile([128, 16], f32)
            nc.gpsimd.tensor_tensor(out=R1[:, :], in0=M[:, :], in1=M1, op=SUB)
            nc.gpsimd.tensor_tensor(out=M3, in0=R1[:, :], in1=M2, op=SUB)

            # Staging tiles [128 (i), (ci, kl, r)] with r padded 9->32 so the
            # transposed rows land at partition base 32*kl (matmul operands
            # must sit at base partition 0/32/64/96; this also spreads the 4
            # joints across PE row-groups so matmuls/LDWEIGHTS overlap).
            WL = sb.tile([128, 512], bf16)
            WR = sb.tile([128, 512], bf16)
            nc.vector.memset(WL[:, :], 0.5)
            nc.gpsimd.memset(WR[:, :], 0.5)
            WLv = WL.rearrange("p (ci kl r) -> p ci kl r", ci=4, r=32)
            WRv = WR.rearrange("p (ci kl r) -> p ci kl r", ci=4, r=32)
            DBv = DB.rearrange("p (ci kl c) -> p ci kl c", ci=4, c=3)
            # WL on DVE, WR on GpSimd — the two chains run in parallel
            nc.vector.tensor_copy(WLv[:, :, :, 3:6], DBv[:, :, :, :])
            nc.gpsimd.tensor_copy(WRv[:, :, :, 3:6], DBv[:, :, :, :])
            # all three m-split cols in one strided copy per side (WR's on
            # the otherwise idle ScalarE; gpsimd's strided copy is slower)
            M123v = M123.rearrange("p (j ci kl) -> p ci kl j", ci=4, kl=4)
            nc.vector.tensor_copy(WLv[:, :, :, 6:9], M123v[:, :, :, :])
            nc.scalar.copy(WRv[:, :, :, 0:3], M123v[:, :, :, :])

            ident = sb.tile([128, 128], bf16)
            make_identity(nc, ident[:, :])

            # Two PSUM tiles in different banks (a single bank would make
            # Tile's bank-overlap tracker serialize the ACT and DVE readers).
            # PSR first: its pool slot frees first, so G(k0) allocates early.
            PSR = ps.tile([128, 512], bf16, tag="g")
            PSL = ps.tile([128, 512], bf16, tag="g")
            for ci in range(4):
                nc.tensor.transpose(
                    PSR[:, 128 * ci:128 * ci + 128], WR[:, 128 * ci:128 * ci + 128], ident[:, :]
                )
                nc.tensor.transpose(
                    PSL[:, 128 * ci:128 * ci + 128], WL[:, 128 * ci:128 * ci + 128], ident[:, :]
                )
            # SR on DVE (reads PSR's banks) in parallel with SL on ScalarE
            # (reads PSL's banks — disjoint, so no bank-overlap serializing);
            # SLq derives from PSL directly on DVE so it doesn't wait for SL.
            SR = sb.tile([128, 512], bf16)
            SL = sb.tile([128, 512], bf16)
            SLq = sb.tile([128, 512], bf16)   # 0.25 * SL (exact in bf16)
            nc.vector.tensor_copy(SR[:, :], PSR[:, :])
            nc.vector.tensor_copy(SL[:, :], PSL[:, :])
            nc.vector.tensor_scalar(
                out=SLq[:, :], in0=SL[:, :], scalar1=0.25, scalar2=None,
                op0=mybir.AluOpType.mult,
            )

            # Per joint: upper-triangle blocks packed into [128, 1280] with
            # the 4 (half-scaled) diagonal blocks contiguous at cols 0:512 and
            # off-diagonal regions at [512(384w), 896(128w), 1024(256w)] —
            # every matmul output stays inside one 2KB PSUM bank, and the
            # diag half is a contiguous AP so joint 0's sqrt can start right
            # after the 4 diag matmuls (shorter ACT pipeline fill).
            OFF0 = (512, 1024, 896)  # off-diag col starts for ci = 0, 1, 2
            ACC = sb.tile([128, J_LOC + 1], f32)
            for kl in range(J_LOC):
                G = ps.tile([128, 1280], f32, tag="g")
                r0 = 32 * kl
                # off-diagonal matmuls first: they only need SL/SR, which
                # are ready one DVE op before SLq
                for ci in range(3):
                    col = OFF0[ci]
                    nc.tensor.matmul(
                        G[:, col:col + 384 - 128 * ci],
                        lhsT=SL[r0:r0 + 9, 128 * ci:128 * ci + 128],
                        rhs=SR[r0:r0 + 9, 128 * (ci + 1):512],
                        start=True, stop=True, tile_position=(r0, 0),
                    )
                for ci in range(4):
                    nc.tensor.matmul(
                        G[:, 128 * ci:128 * ci + 128],
                        lhsT=SLq[r0:r0 + 9, 128 * ci:128 * ci + 128],
                        rhs=SR[r0:r0 + 9, 128 * ci:128 * ci + 128],
                        start=True, stop=True, tile_position=(r0, 0),
                    )
                # deform = sqrt(-2*h + EPS); accum_out = per-partition sum.
                # Joint 0 is split off-diag/diag so ACT starts ~600ns earlier
                # (its first chunk only waits on the 3 off-diag matmuls).
                if kl == 0:
                    nc.scalar.activation(
                        out=G[:, 512:1280], in_=G[:, 512:1280],
                        func=mybir.ActivationFunctionType.Sqrt,
                        bias=eps_t[:, :], scale=-2.0,
                        accum_out=ACC[:, 0:1],
                    )
                    nc.scalar.activation(
                        out=G[:, 0:512], in_=G[:, 0:512],
                        func=mybir.ActivationFunctionType.Sqrt,
                        bias=eps_t[:, :], scale=-2.0,
                        accum_out=ACC[:, 1:2],
                    )
                else:
                    nc.scalar.activation(
                        out=G[:, :], in_=G[:, :],
                        func=mybir.ActivationFunctionType.Sqrt,
                        bias=eps_t[:, :], scale=-2.0,
                        accum_out=ACC[:, kl + 1:kl + 2],
                    )
            nc.sync.dma_start(out=acc_out[:, :], in_=ACC[:, :])

    return nc


def make_in_maps(pred_3d: np.ndarray, reconstructed_3d: np.ndarray):
    """Shard: core c gets joints [4c, 4c+4), packed as [128, 96] f32 with
    x[p, t*48 + ci*12 + kl*3 + c] = (pred,recon)[128*ci + p, 4*cc + kl, c]."""
    pred = np.asarray(pred_3d, dtype=np.float32)
    recon = np.asarray(reconstructed_3d, dtype=np.float32)
    in_maps = []
    for cc in range(NCORES):
        sl = slice(J_LOC * cc, J_LOC * cc + J_LOC)
        arr = np.stack([pred[:, sl, :], recon[:, sl, :]])  # [2, 512, 4, 3]
        arr = (
            arr.reshape(2, 4, 128, J_LOC * 3)
            .transpose(2, 0, 1, 3)
            .reshape(128, 96)
        )
        in_maps.append({"x": np.ascontiguousarray(arr)})
    return in_maps


def _get_nc():
    if "nc" not in _STATE:
        _STATE["nc"] = build_bass()
    return _STATE["nc"]


def reduce_outputs(results) -> np.ndarray:
    total = np.float64(0.0)
    for r in results:
        total += np.asarray(r["acc"], dtype=np.float64).sum()
    # computed = diag/2 + upper-off-diag; full-matrix total = 2 * computed
    loss = 2.0 * total / (2.0 * J * NUM_PAIRS)
    return np.float32(loss)


def kernel(pred_3d: np.ndarray, reconstructed_3d: np.ndarray) -> np.ndarray:
    _ensure_path()
    install_walrus_wait_split()
    from concourse.bass_utils import run_bass_kernel_spmd

    nc = _get_nc()
    in_maps = make_in_maps(pred_3d, reconstructed_3d)
    res = run_bass_kernel_spmd(nc, in_maps, list(range(NCORES)))
    return reduce_outputs(res.results)

